# revision 24
# baseline (speedup 1.0000x reference)
"""Trainium2 Bass kernel: multi-head attention (B=2, T=2048, C=2048, H=16, D=128).

Sharding: tensor-parallel over heads. 8 cores x 2 heads each.
  - W_qkv columns sliced per head-pair, W_proj rows sliced per head-pair.
  - Each core computes a partial output [B*T, C]; host sums the 8 partials
    (the standard row-parallel unshard).

v2: cross-phase overlap. All pools are persistent (opened once) with enough
buffers that the Tile list-scheduler can fill PE gaps in the ScalarE-bound
attention phase of batch b with projection MMs of batch b+1 / out-projection
MMs of batch b-1:
  emission order: proj(b0), attn(b0), proj(b1), outproj(b0), attn(b1),
  outproj(b1); priorities follow emission, deps allow interleave.

Attention: scores tiles are [128, 1024] f32 (two PSUM banks, written by two
N=512 matmuls) so exp runs as one [128,1024] ScalarE activation (amortizes
the ~350-cycle ACT overhead). Softmax denominator: dacc accumulates exp
tiles on DVE (bf16, 2x mode); finalize = gpsimd partition_all_reduce (sums
over key partitions AND broadcasts in one op), reciprocal_approx_fast on
DVE, then one [128,1024] normalize mul.

PSUM budget (8 banks): proj q/k/v 3 + scores 2 + y 2 = 7. Out-projection
tiles rotate through the proj q/k/v tags (disjoint lifetime).
"""

import math

import numpy as np

N_CORES = 8
B, T, C = 2, 2048, 2048
N_HEAD, D = 16, 128
HPC = N_HEAD // N_CORES          # heads per core
JC = HPC * D                     # per-core slice width of qkv/proj dims

RP = 256                         # proj token tile (q/k packed 2 heads/bank)
QT = 512                         # attn query tile (one PSUM bank)
KB = 128                         # key block (contraction tile) in attention

# filled by _build: list of (label, first_unused_inst_id)
PHASE_MARKS = []


def _build(Bp, Tp, Cp, hpc, d):
    """Build the per-core Bass graph. All cores run the same graph on
    different weight slices."""
    PHASE_MARKS.clear()
    import concourse.bacc as bacc
    import concourse.tile as tile
    from concourse import mybir

    f32 = mybir.dt.float32
    bf16 = mybir.dt.bfloat16
    Exp = mybir.ActivationFunctionType.Exp

    jc = hpc * d
    BT = Bp * Tp
    n_ck = Cp // 128             # contraction chunks for proj
    n_rp = Tp // RP              # proj token tiles per batch
    n_qt = Tp // QT              # attn query tiles per batch
    n_kb = Tp // KB              # key blocks per batch
    n_rb = Tp // 128             # row blocks for out proj
    n_ot = Cp // 512             # output column tiles
    n_sub = RP // 128
    scale = 1.0 / math.sqrt(d)
    hd = d // 2

    nc = bacc.Bacc("TRN2", target_bir_lowering=False, debug=False)

    xT = nc.declare_dram_parameter("xT", [Cp, BT], bf16, isOutput=False)
    wqkv = nc.declare_dram_parameter("wqkv", [Cp, 3 * jc], bf16,
                                     isOutput=False)
    wp = nc.declare_dram_parameter("wp", [jc, Cp], bf16, isOutput=False)
    cosT = nc.declare_dram_parameter("cosT", [d, Tp], f32, isOutput=False)
    sinT = nc.declare_dram_parameter("sinT", [d, Tp], f32, isOutput=False)
    out = nc.declare_dram_parameter("out", [BT, Cp], bf16, isOutput=True)

    with tile.TileContext(nc) as tc:
        with (
            nc.allow_low_precision(reason="bf16 accumulation audited"),
            tc.tile_pool(name="wpool", bufs=1) as wpool,
            tc.tile_pool(name="xpool", bufs=4) as xpool,
            tc.tile_pool(name="acts", bufs=2) as acts,
            tc.tile_pool(name="rope", bufs=4) as rope,
            tc.tile_pool(name="epool", bufs=8) as epool,
            tc.tile_pool(name="dpool", bufs=2) as dpool,
            tc.tile_pool(name="spool", bufs=2) as spool,
            tc.tile_pool(name="opool", bufs=3) as opool,
            tc.tile_pool(name="pproj", bufs=1, space="PSUM") as pproj,
            tc.tile_pool(name="pscore", bufs=3, space="PSUM") as pscore,
            tc.tile_pool(name="pyy", bufs=2, space="PSUM") as pyy,
        ):
            # ---- resident weights / tables ----
            # ramp-critical order: w0 + the first token tile first, so the
            # first proj matmuls start as soon as ~1.2MB has landed; the
            # remaining weight chunks stream in at matmul-consumption rate
            w_tiles = [wpool.tile([128, 3 * jc], bf16, tag=f"w{ck}",
                                  name=f"w{ck}") for ck in range(n_ck)]
            wq_sb = [t[:, 0:jc] for t in w_tiles]
            wk_sb = [t[:, jc:2 * jc] for t in w_tiles]
            wv_sb = [t[:, 2 * jc:3 * jc] for t in w_tiles]
            for ck in range(n_ck):
                nc.sync.dma_start(w_tiles[ck],
                                  wqkv[ck * 128:(ck + 1) * 128, :])
            xt_pre = xpool.tile([128, n_ck, RP], bf16, tag="xt",
                                name="xtpre")
            nc.sync.dma_start(
                xt_pre, xT[:, 0:RP].rearrange("(n p) t -> p n t", p=128))
            warm = wpool.tile([128, 8], f32, tag="warm")
            nc.vector.memset(warm[:, 0:4], 0.0)
            nc.scalar.activation(warm[:, 4:8], warm[:, 0:4], Exp)
            ones_bf = wpool.tile([128, 1], bf16, tag="ones_bf")
            nc.vector.memset(ones_bf, 1.0)
            HT = 2 * RP
            cos_a = wpool.tile([d, Tp], f32, tag="cos_a")
            sin_a = wpool.tile([d, Tp], f32, tag="sin_a")
            nc.sync.dma_start(cos_a, cosT[:])
            nc.sync.dma_start(sin_a, sinT[:])
            cos_b, sin_b = cos_a, sin_a
            wp_sb = wpool.tile([128, hpc, Cp], bf16, tag="wp")
            nc.sync.dma_start(wp_sb, wp.rearrange("(h p) o -> p h o", p=128))

            def cos_sin(tsl):
                return cos_a[:, tsl], sin_a[:, tsl]

            # per-batch activation tiles (bufs=2 -> both batches coexist)
            def batch_tiles(b):
                return (
                    acts.tile([128, hpc, Tp], bf16, tag="qT", name=f"qT{b}"),
                    acts.tile([128, hpc, Tp], bf16, tag="kT", name=f"kT{b}"),
                    acts.tile([128, n_kb, jc], bf16, tag="v", name=f"v{b}"),
                    acts.tile([128, hpc, Tp], bf16, tag="yT", name=f"yT{b}"),
                )

            bt = [batch_tiles(0), batch_tiles(1)]

            def proj(b):
                """qkv projection + rope for batch b."""
                PHASE_MARKS.append((f"proj{b}", nc.next_id()))
                qT_sb, kT_sb, v_sb, _ = bt[b]
                for rt in range(n_rp):
                    rsl = slice(b * Tp + rt * RP, b * Tp + (rt + 1) * RP)
                    tsl = slice(rt * RP, (rt + 1) * RP)
                    if b == 0 and rt == 0:
                        xt_big = xt_pre
                    else:
                        xt_big = xpool.tile([128, n_ck, RP], bf16, tag="xt")
                        nc.sync.dma_start(
                            xt_big,
                            xT[:, rsl].rearrange("(n p) t -> p n t", p=128))
                    xts = [xt_big[:, ck, :] for ck in range(n_ck)]
                    cos_t, sin_t = cos_sin(tsl)

                    def do_rope(ps, dst, h):
                        # dst = psum*cos + swap(psum)*sin_signed
                        t1 = rope.tile([d, RP], f32, tag="t1")
                        nc.vector.tensor_mul(t1, ps, cos_t)
                        t2 = rope.tile([d, RP], f32, tag="t2")
                        nc.vector.tensor_mul(
                            t2[0:hd], ps[hd:d], sin_t[0:hd])
                        nc.vector.tensor_mul(
                            t2[hd:d], ps[0:hd], sin_t[hd:d])
                        nc.vector.tensor_add(dst[:, h, tsl], t1, t2)

                    # three passes (q | k | v) so the rope/copy epilogue of
                    # each accumulator drains while the next pass's matmuls
                    # run: no PE stall even with single-buffered psum tags
                    q_ps = pproj.tile([128, hpc * RP], f32, tag="qps")
                    for ck in range(n_ck):
                        for h in range(hpc):
                            nc.tensor.matmul(
                                q_ps[:, h * RP:(h + 1) * RP],
                                wq_sb[ck][:, h * d:(h + 1) * d],
                                xts[ck], start=(ck == 0 and h == 0),
                                stop=(ck == n_ck - 1 and h == hpc - 1),
                                skip_group_check=True)
                    for h in range(hpc):
                        do_rope(q_ps[:, h * RP:(h + 1) * RP], qT_sb, h)
                    k_ps = pproj.tile([128, hpc * RP], f32, tag="kps")
                    for ck in range(n_ck):
                        for h in range(hpc):
                            nc.tensor.matmul(
                                k_ps[:, h * RP:(h + 1) * RP],
                                wk_sb[ck][:, h * d:(h + 1) * d],
                                xts[ck], start=(ck == 0 and h == 0),
                                stop=(ck == n_ck - 1 and h == hpc - 1),
                                skip_group_check=True)
                    for h in range(hpc):
                        do_rope(k_ps[:, h * RP:(h + 1) * RP], kT_sb, h)
                    v_ps = pproj.tile([128, n_sub * jc], f32, tag="vps")
                    for ck in range(n_ck):
                        for s in range(n_sub):
                            nc.tensor.matmul(
                                v_ps[:, s * jc:(s + 1) * jc],
                                xts[ck][:, s * 128:(s + 1) * 128],
                                wv_sb[ck], start=(ck == 0 and s == 0),
                                stop=(ck == n_ck - 1 and s == n_sub - 1),
                                skip_group_check=True)
                    nc.scalar.copy(
                        v_sb[:, rt * n_sub:(rt + 1) * n_sub, :], v_ps)

            def attn(b):
                """attention for batch b: softmax(qk^T/sqrt(d)) v."""
                PHASE_MARKS.append((f"attn{b}", nc.next_id()))
                qT_sb, kT_sb, v_sb, yT_sb = bt[b]
                for qt in range(n_qt):
                    for h in range(hpc):
                        qsl = slice(qt * QT, (qt + 1) * QT)
                        dacc = dpool.tile([128, QT], bf16, tag="dacc",
                                          name=f"dacc{b}{qt}{h}")
                        y_ps = pyy.tile([128, QT], f32, tag="y",
                                        name=f"y{b}{qt}{h}")
                        for kb in range(n_kb):
                            s_ps = pscore.tile([128, QT], f32, tag="s")
                            kT_blk = kT_sb[:, h, kb * KB:(kb + 1) * KB]
                            nc.tensor.matmul(
                                s_ps, kT_blk, qT_sb[:, h, qsl],
                                start=True, stop=True,
                                skip_group_check=True)
                            e_sb = epool.tile([128, QT], bf16, tag="e")
                            nc.scalar.activation(e_sb, s_ps, Exp, scale=scale)
                            if kb == 0:
                                nc.vector.tensor_copy(out=dacc, in_=e_sb)
                            else:
                                nc.vector.tensor_add(dacc, dacc, e_sb)
                            v_blk = v_sb[:, kb, h * d:(h + 1) * d]
                            nc.tensor.matmul(
                                y_ps, v_blk, e_sb,
                                start=(kb == 0), stop=(kb == n_kb - 1),
                                skip_group_check=True)
                        # finalize: den[q] = ones^T @ dacc (PE, shares the
                        # scores psum tag); 1/den on DVE (fast approx);
                        # broadcast over partitions on gpsimd; normalize
                        dsum_ps = pscore.tile([1, QT], f32, tag="s",
                                              name=f"dsum{b}{qt}{h}")
                        nc.tensor.matmul(dsum_ps, ones_bf, dacc,
                                         start=True, stop=True,
                                         skip_group_check=True)
                        recip = spool.tile([1, QT], f32, tag="recip")
                        nc.vector.reciprocal_approx_fast(
                            out=recip, in_=dsum_ps)
                        bcr = spool.tile([128, QT], f32, tag="bcr")
                        nc.gpsimd.partition_broadcast(bcr, recip)
                        nc.vector.tensor_mul(yT_sb[:, h, qsl], y_ps, bcr)

            def outproj(b):
                """out projection for batch b -> partial DRAM output.
                PSUM tiles rotate through the proj pool tags (their
                lifetimes do not overlap proj use of the same banks)."""
                PHASE_MARKS.append((f"outproj{b}", nc.next_id()))
                yT_sb = bt[b][3]
                otags = ("qps", "kps", "vps")
                for rb in range(n_rb):
                    o_sb = opool.tile([128, Cp], bf16, tag="o")
                    for ot in range(n_ot):
                        o_ps = pproj.tile([128, hpc * RP], f32,
                                          tag=otags[(rb * n_ot + ot) % 3])
                        for h in range(hpc):
                            nc.tensor.matmul(
                                o_ps[:, 0:512],
                                yT_sb[:, h, rb * 128:(rb + 1) * 128],
                                wp_sb[:, h, ot * 512:(ot + 1) * 512],
                                start=(h == 0), stop=(h == hpc - 1),
                                skip_group_check=True)
                        osl = o_sb[:, ot * 512:(ot + 1) * 512]
                        if (rb * n_ot + ot) % 2 == 0:
                            nc.vector.tensor_copy(out=osl, in_=o_ps[:, 0:512])
                        else:
                            nc.scalar.copy(osl, o_ps[:, 0:512])
                    # one contiguous-row DMA per 128-token block (4KB lines)
                    nc.sync.dma_start(
                        out[b * Tp + rb * 128:b * Tp + (rb + 1) * 128, :],
                        o_sb)

            proj(0)
            attn(0)
            proj(1)
            attn(1)
            outproj(0)
            outproj(1)

    PHASE_MARKS.append(("tail", nc.next_id()))
    nc.compile()
    return nc


def _prep_in_maps(x, cos, sin, W_qkv, W_proj, n_cores, hpc, d):
    """Host-side shard prep: pure layout work (transpose / slice / sign fold)."""
    Bp, Tp, Cp = x.shape
    jc = hpc * d
    import ml_dtypes
    xTa = np.ascontiguousarray(x.reshape(Bp * Tp, Cp).T).astype(ml_dtypes.bfloat16)
    cosT = np.ascontiguousarray(cos.T)
    sinT = np.ascontiguousarray(sin.T).copy()
    sinT[: d // 2] *= -1.0
    in_maps = []
    for c in range(n_cores):
        j0, j1 = c * jc, (c + 1) * jc
        in_maps.append({
            "xT": xTa,
            "wqkv": np.ascontiguousarray(np.concatenate(
                [W_qkv[:, j0:j1], W_qkv[:, Cp + j0:Cp + j1],
                 W_qkv[:, 2 * Cp + j0:2 * Cp + j1]], axis=1,
            )).astype(ml_dtypes.bfloat16),
            "wp": np.ascontiguousarray(W_proj[j0:j1, :]).astype(ml_dtypes.bfloat16),
            "cosT": cosT,
            "sinT": sinT,
        })
    return in_maps


def _install_ntff_hook():
    """Enable NTFF profiling under axon when the boot image lacks the
    antenv.axon_hooks shim. Harmless if anything is missing."""
    import sys
    import types
    try:
        from antenv.axon_hooks import get_axon_ntff_profile_hook
        if get_axon_ntff_profile_hook() is not None:
            return
    except ImportError:
        pass
    try:
        sys.path.insert(0, "/root/.axon_site")
        from trn_agent_boot.trn_boot import _ntff_profile_via_ctypes

        hook = _ntff_profile_via_ctypes("/opt/axon/libaxon_pjrt.so")
        if hook is None:
            return
        mod = types.ModuleType("antenv.axon_hooks")
        mod.get_axon_ntff_profile_hook = lambda: hook
        mod.set_axon_ntff_profile_hook = lambda h: None
        import antenv
        antenv.axon_hooks = mod
        sys.modules["antenv.axon_hooks"] = mod
    except Exception:
        pass


def _run(x, cos, sin, W_qkv, W_proj, trace=False):
    from concourse.bass_utils import run_bass_kernel_spmd

    if trace:
        _install_ntff_hook()

    x = np.ascontiguousarray(x, dtype=np.float32)
    cos = np.ascontiguousarray(cos, dtype=np.float32)
    sin = np.ascontiguousarray(sin, dtype=np.float32)
    W_qkv = np.ascontiguousarray(W_qkv, dtype=np.float32)
    W_proj = np.ascontiguousarray(W_proj, dtype=np.float32)

    Bp, Tp, Cp = x.shape
    nc = _build(Bp, Tp, Cp, HPC, D)
    in_maps = _prep_in_maps(x, cos, sin, W_qkv, W_proj, N_CORES, HPC, D)
    res = run_bass_kernel_spmd(nc, in_maps, core_ids=list(range(N_CORES)),
                               trace=trace)
    acc = np.zeros((Bp * Tp, Cp), dtype=np.float32)
    for i in range(N_CORES):
        acc += np.asarray(res.results[i]["out"], dtype=np.float32)
    return acc.reshape(Bp, Tp, Cp), res


def kernel(x, cos, sin, W_qkv, W_proj):
    out, _ = _run(x, cos, sin, W_qkv, W_proj, trace=False)
    return out


# revision 25
# speedup vs baseline: 1.0085x; 1.0085x over previous
"""Trainium2 Bass kernel: multi-head attention (B=2, T=2048, C=2048, H=16, D=128).

Sharding: tensor-parallel over heads. 8 cores x 2 heads each.
  - W_qkv columns sliced per head-pair, W_proj rows sliced per head-pair.
  - Each core computes a partial output [B*T, C]; host sums the 8 partials
    (the standard row-parallel unshard).

v2: cross-phase overlap. All pools are persistent (opened once) with enough
buffers that the Tile list-scheduler can fill PE gaps in the ScalarE-bound
attention phase of batch b with projection MMs of batch b+1 / out-projection
MMs of batch b-1:
  emission order: proj(b0), attn(b0), proj(b1), outproj(b0), attn(b1),
  outproj(b1); priorities follow emission, deps allow interleave.

Attention: scores tiles are [128, 1024] f32 (two PSUM banks, written by two
N=512 matmuls) so exp runs as one [128,1024] ScalarE activation (amortizes
the ~350-cycle ACT overhead). Softmax denominator: dacc accumulates exp
tiles on DVE (bf16, 2x mode); finalize = gpsimd partition_all_reduce (sums
over key partitions AND broadcasts in one op), reciprocal_approx_fast on
DVE, then one [128,1024] normalize mul.

PSUM budget (8 banks): proj q/k/v 3 + scores 2 + y 2 = 7. Out-projection
tiles rotate through the proj q/k/v tags (disjoint lifetime).
"""

import math

import numpy as np

N_CORES = 8
B, T, C = 2, 2048, 2048
N_HEAD, D = 16, 128
HPC = N_HEAD // N_CORES          # heads per core
JC = HPC * D                     # per-core slice width of qkv/proj dims

RP = 256                         # proj token tile (q/k packed 2 heads/bank)
QT = 512                         # attn query tile (one PSUM bank)
KB = 128                         # key block (contraction tile) in attention

# filled by _build: list of (label, first_unused_inst_id)
PHASE_MARKS = []


def _build(Bp, Tp, Cp, hpc, d):
    """Build the per-core Bass graph. All cores run the same graph on
    different weight slices."""
    PHASE_MARKS.clear()
    import concourse.bacc as bacc
    import concourse.tile as tile
    from concourse import mybir

    f32 = mybir.dt.float32
    bf16 = mybir.dt.bfloat16
    Exp = mybir.ActivationFunctionType.Exp

    jc = hpc * d
    BT = Bp * Tp
    n_ck = Cp // 128             # contraction chunks for proj
    n_rp = Tp // RP              # proj token tiles per batch
    n_qt = Tp // QT              # attn query tiles per batch
    n_kb = Tp // KB              # key blocks per batch
    n_rb = Tp // 128             # row blocks for out proj
    n_ot = Cp // 512             # output column tiles
    n_sub = RP // 128
    scale = 1.0 / math.sqrt(d)
    hd = d // 2

    nc = bacc.Bacc("TRN2", target_bir_lowering=False, debug=False)

    xT = nc.declare_dram_parameter("xT", [Cp, BT], bf16, isOutput=False)
    wqkv = nc.declare_dram_parameter("wqkv", [Cp, 3 * jc], bf16,
                                     isOutput=False)
    wp = nc.declare_dram_parameter("wp", [jc, Cp], bf16, isOutput=False)
    cosT = nc.declare_dram_parameter("cosT", [d, Tp], f32, isOutput=False)
    sinT = nc.declare_dram_parameter("sinT", [d, Tp], f32, isOutput=False)
    out = nc.declare_dram_parameter("out", [BT, Cp], bf16, isOutput=True)

    with tile.TileContext(nc) as tc:
        with (
            nc.allow_low_precision(reason="bf16 accumulation audited"),
            tc.tile_pool(name="wpool", bufs=1) as wpool,
            tc.tile_pool(name="xpool", bufs=4) as xpool,
            tc.tile_pool(name="acts", bufs=2) as acts,
            tc.tile_pool(name="rope", bufs=4) as rope,
            tc.tile_pool(name="epool", bufs=8) as epool,
            tc.tile_pool(name="dpool", bufs=2) as dpool,
            tc.tile_pool(name="spool", bufs=2) as spool,
            tc.tile_pool(name="opool", bufs=3) as opool,
            tc.tile_pool(name="pproj", bufs=1, space="PSUM") as pproj,
            tc.tile_pool(name="pscore", bufs=3, space="PSUM") as pscore,
            tc.tile_pool(name="pyy", bufs=2, space="PSUM") as pyy,
        ):
            # ---- resident weights / tables ----
            # ramp-critical order: w0 + the first token tile first, so the
            # first proj matmuls start as soon as ~1.2MB has landed; the
            # remaining weight chunks stream in at matmul-consumption rate
            w_big = wpool.tile([128, n_ck, 3 * jc], bf16, tag="wbig")
            nc.sync.dma_start(w_big,
                              wqkv.rearrange("(n p) j -> p n j", p=128))
            wq_sb = [w_big[:, ck, 0:jc] for ck in range(n_ck)]
            wk_sb = [w_big[:, ck, jc:2 * jc] for ck in range(n_ck)]
            wv_sb = [w_big[:, ck, 2 * jc:3 * jc] for ck in range(n_ck)]
            xt_pre = xpool.tile([128, n_ck, RP], bf16, tag="xt",
                                name="xtpre")
            nc.sync.dma_start(
                xt_pre, xT[:, 0:RP].rearrange("(n p) t -> p n t", p=128))
            warm = wpool.tile([128, 8], f32, tag="warm")
            nc.vector.memset(warm[:, 0:4], 0.0)
            nc.scalar.activation(warm[:, 4:8], warm[:, 0:4], Exp)
            ones_bf = wpool.tile([128, 1], bf16, tag="ones_bf")
            nc.vector.memset(ones_bf, 1.0)
            HT = 2 * RP
            cos_a = wpool.tile([d, Tp], f32, tag="cos_a")
            sin_a = wpool.tile([d, Tp], f32, tag="sin_a")
            nc.sync.dma_start(cos_a, cosT[:])
            nc.sync.dma_start(sin_a, sinT[:])
            cos_b, sin_b = cos_a, sin_a
            wp_sb = wpool.tile([128, hpc, Cp], bf16, tag="wp")
            nc.sync.dma_start(wp_sb, wp.rearrange("(h p) o -> p h o", p=128))

            def cos_sin(tsl):
                return cos_a[:, tsl], sin_a[:, tsl]

            # per-batch activation tiles (bufs=2 -> both batches coexist)
            def batch_tiles(b):
                return (
                    acts.tile([128, hpc, Tp], bf16, tag="qT", name=f"qT{b}"),
                    acts.tile([128, hpc, Tp], bf16, tag="kT", name=f"kT{b}"),
                    acts.tile([128, n_kb, jc], bf16, tag="v", name=f"v{b}"),
                    acts.tile([128, hpc, Tp], bf16, tag="yT", name=f"yT{b}"),
                )

            bt = [batch_tiles(0), batch_tiles(1)]

            def proj(b):
                """qkv projection + rope for batch b."""
                PHASE_MARKS.append((f"proj{b}", nc.next_id()))
                qT_sb, kT_sb, v_sb, _ = bt[b]
                for rt in range(n_rp):
                    rsl = slice(b * Tp + rt * RP, b * Tp + (rt + 1) * RP)
                    tsl = slice(rt * RP, (rt + 1) * RP)
                    if b == 0 and rt == 0:
                        xt_big = xt_pre
                    else:
                        xt_big = xpool.tile([128, n_ck, RP], bf16, tag="xt")
                        nc.sync.dma_start(
                            xt_big,
                            xT[:, rsl].rearrange("(n p) t -> p n t", p=128))
                    xts = [xt_big[:, ck, :] for ck in range(n_ck)]
                    cos_t, sin_t = cos_sin(tsl)

                    def do_rope(ps, dst, h):
                        # dst = psum*cos + swap(psum)*sin_signed
                        t1 = rope.tile([d, RP], f32, tag="t1")
                        nc.vector.tensor_mul(t1, ps, cos_t)
                        t2 = rope.tile([d, RP], f32, tag="t2")
                        nc.vector.tensor_mul(
                            t2[0:hd], ps[hd:d], sin_t[0:hd])
                        nc.vector.tensor_mul(
                            t2[hd:d], ps[0:hd], sin_t[hd:d])
                        nc.vector.tensor_add(dst[:, h, tsl], t1, t2)

                    # three passes (q | k | v) so the rope/copy epilogue of
                    # each accumulator drains while the next pass's matmuls
                    # run: no PE stall even with single-buffered psum tags
                    q_ps = pproj.tile([128, hpc * RP], f32, tag="qps")
                    for ck in range(n_ck):
                        for h in range(hpc):
                            nc.tensor.matmul(
                                q_ps[:, h * RP:(h + 1) * RP],
                                wq_sb[ck][:, h * d:(h + 1) * d],
                                xts[ck], start=(ck == 0 and h == 0),
                                stop=(ck == n_ck - 1 and h == hpc - 1),
                                skip_group_check=True)
                    for h in range(hpc):
                        do_rope(q_ps[:, h * RP:(h + 1) * RP], qT_sb, h)
                    k_ps = pproj.tile([128, hpc * RP], f32, tag="kps")
                    for ck in range(n_ck):
                        for h in range(hpc):
                            nc.tensor.matmul(
                                k_ps[:, h * RP:(h + 1) * RP],
                                wk_sb[ck][:, h * d:(h + 1) * d],
                                xts[ck], start=(ck == 0 and h == 0),
                                stop=(ck == n_ck - 1 and h == hpc - 1),
                                skip_group_check=True)
                    for h in range(hpc):
                        do_rope(k_ps[:, h * RP:(h + 1) * RP], kT_sb, h)
                    v_ps = pproj.tile([128, n_sub * jc], f32, tag="vps")
                    for ck in range(n_ck):
                        for s in range(n_sub):
                            nc.tensor.matmul(
                                v_ps[:, s * jc:(s + 1) * jc],
                                xts[ck][:, s * 128:(s + 1) * 128],
                                wv_sb[ck], start=(ck == 0 and s == 0),
                                stop=(ck == n_ck - 1 and s == n_sub - 1),
                                skip_group_check=True)
                    nc.scalar.copy(
                        v_sb[:, rt * n_sub:(rt + 1) * n_sub, :], v_ps)

            def attn(b):
                """attention for batch b: softmax(qk^T/sqrt(d)) v."""
                PHASE_MARKS.append((f"attn{b}", nc.next_id()))
                qT_sb, kT_sb, v_sb, yT_sb = bt[b]
                for qt in range(n_qt):
                    for h in range(hpc):
                        qsl = slice(qt * QT, (qt + 1) * QT)
                        dacc = dpool.tile([128, QT], bf16, tag="dacc",
                                          name=f"dacc{b}{qt}{h}")
                        y_ps = pyy.tile([128, QT], f32, tag="y",
                                        name=f"y{b}{qt}{h}")
                        for kb in range(n_kb):
                            s_ps = pscore.tile([128, QT], f32, tag="s")
                            kT_blk = kT_sb[:, h, kb * KB:(kb + 1) * KB]
                            nc.tensor.matmul(
                                s_ps, kT_blk, qT_sb[:, h, qsl],
                                start=True, stop=True,
                                skip_group_check=True)
                            e_sb = epool.tile([128, QT], bf16, tag="e")
                            nc.scalar.activation(e_sb, s_ps, Exp, scale=scale)
                            if kb == 0:
                                nc.vector.tensor_copy(out=dacc, in_=e_sb)
                            else:
                                nc.vector.tensor_add(dacc, dacc, e_sb)
                            v_blk = v_sb[:, kb, h * d:(h + 1) * d]
                            nc.tensor.matmul(
                                y_ps, v_blk, e_sb,
                                start=(kb == 0), stop=(kb == n_kb - 1),
                                skip_group_check=True)
                        # finalize: den[q] = ones^T @ dacc (PE, shares the
                        # scores psum tag); 1/den on DVE (fast approx);
                        # broadcast over partitions on gpsimd; normalize
                        dsum_ps = pscore.tile([1, QT], f32, tag="s",
                                              name=f"dsum{b}{qt}{h}")
                        nc.tensor.matmul(dsum_ps, ones_bf, dacc,
                                         start=True, stop=True,
                                         skip_group_check=True)
                        recip = spool.tile([1, QT], f32, tag="recip")
                        nc.vector.reciprocal_approx_fast(
                            out=recip, in_=dsum_ps)
                        bcr = spool.tile([128, QT], f32, tag="bcr")
                        nc.gpsimd.partition_broadcast(bcr, recip)
                        nc.vector.tensor_mul(yT_sb[:, h, qsl], y_ps, bcr)

            def outproj(b):
                """out projection for batch b -> partial DRAM output.
                PSUM tiles rotate through the proj pool tags (their
                lifetimes do not overlap proj use of the same banks)."""
                PHASE_MARKS.append((f"outproj{b}", nc.next_id()))
                yT_sb = bt[b][3]
                otags = ("qps", "kps", "vps")
                for rb in range(n_rb):
                    o_sb = opool.tile([128, Cp], bf16, tag="o")
                    for ot in range(n_ot):
                        o_ps = pproj.tile([128, hpc * RP], f32,
                                          tag=otags[(rb * n_ot + ot) % 3])
                        for h in range(hpc):
                            nc.tensor.matmul(
                                o_ps[:, 0:512],
                                yT_sb[:, h, rb * 128:(rb + 1) * 128],
                                wp_sb[:, h, ot * 512:(ot + 1) * 512],
                                start=(h == 0), stop=(h == hpc - 1),
                                skip_group_check=True)
                        osl = o_sb[:, ot * 512:(ot + 1) * 512]
                        if (rb * n_ot + ot) % 2 == 0:
                            nc.vector.tensor_copy(out=osl, in_=o_ps[:, 0:512])
                        else:
                            nc.scalar.copy(osl, o_ps[:, 0:512])
                    # one contiguous-row DMA per 128-token block (4KB lines)
                    nc.sync.dma_start(
                        out[b * Tp + rb * 128:b * Tp + (rb + 1) * 128, :],
                        o_sb)

            proj(0)
            attn(0)
            proj(1)
            outproj(0)
            attn(1)
            outproj(1)

    PHASE_MARKS.append(("tail", nc.next_id()))
    nc.compile()
    return nc


def _prep_in_maps(x, cos, sin, W_qkv, W_proj, n_cores, hpc, d):
    """Host-side shard prep: pure layout work (transpose / slice / sign fold)."""
    Bp, Tp, Cp = x.shape
    jc = hpc * d
    import ml_dtypes
    xTa = np.ascontiguousarray(x.reshape(Bp * Tp, Cp).T).astype(ml_dtypes.bfloat16)
    cosT = np.ascontiguousarray(cos.T)
    sinT = np.ascontiguousarray(sin.T).copy()
    sinT[: d // 2] *= -1.0
    in_maps = []
    for c in range(n_cores):
        j0, j1 = c * jc, (c + 1) * jc
        in_maps.append({
            "xT": xTa,
            "wqkv": np.ascontiguousarray(np.concatenate(
                [W_qkv[:, j0:j1], W_qkv[:, Cp + j0:Cp + j1],
                 W_qkv[:, 2 * Cp + j0:2 * Cp + j1]], axis=1,
            )).astype(ml_dtypes.bfloat16),
            "wp": np.ascontiguousarray(W_proj[j0:j1, :]).astype(ml_dtypes.bfloat16),
            "cosT": cosT,
            "sinT": sinT,
        })
    return in_maps


def _install_ntff_hook():
    """Enable NTFF profiling under axon when the boot image lacks the
    antenv.axon_hooks shim. Harmless if anything is missing."""
    import sys
    import types
    try:
        from antenv.axon_hooks import get_axon_ntff_profile_hook
        if get_axon_ntff_profile_hook() is not None:
            return
    except ImportError:
        pass
    try:
        sys.path.insert(0, "/root/.axon_site")
        from trn_agent_boot.trn_boot import _ntff_profile_via_ctypes

        hook = _ntff_profile_via_ctypes("/opt/axon/libaxon_pjrt.so")
        if hook is None:
            return
        mod = types.ModuleType("antenv.axon_hooks")
        mod.get_axon_ntff_profile_hook = lambda: hook
        mod.set_axon_ntff_profile_hook = lambda h: None
        import antenv
        antenv.axon_hooks = mod
        sys.modules["antenv.axon_hooks"] = mod
    except Exception:
        pass


def _run(x, cos, sin, W_qkv, W_proj, trace=False):
    from concourse.bass_utils import run_bass_kernel_spmd

    if trace:
        _install_ntff_hook()

    x = np.ascontiguousarray(x, dtype=np.float32)
    cos = np.ascontiguousarray(cos, dtype=np.float32)
    sin = np.ascontiguousarray(sin, dtype=np.float32)
    W_qkv = np.ascontiguousarray(W_qkv, dtype=np.float32)
    W_proj = np.ascontiguousarray(W_proj, dtype=np.float32)

    Bp, Tp, Cp = x.shape
    nc = _build(Bp, Tp, Cp, HPC, D)
    in_maps = _prep_in_maps(x, cos, sin, W_qkv, W_proj, N_CORES, HPC, D)
    res = run_bass_kernel_spmd(nc, in_maps, core_ids=list(range(N_CORES)),
                               trace=trace)
    acc = np.zeros((Bp * Tp, Cp), dtype=np.float32)
    for i in range(N_CORES):
        acc += np.asarray(res.results[i]["out"], dtype=np.float32)
    return acc.reshape(Bp, Tp, Cp), res


def kernel(x, cos, sin, W_qkv, W_proj):
    out, _ = _run(x, cos, sin, W_qkv, W_proj, trace=False)
    return out
